# revision 5
# baseline (speedup 1.0000x reference)
"""Multi-head causal attention (B=4, S=2048, D=1024, H=16, hd=64) on 8 trn2 cores.

Sharding: core i handles batch b=i//2 and head-group hg=i%2 (8 heads).
Each core computes partial_out_b = ctx(heads of hg) @ Wo[rows of hg].
Host: out[b] = partial[2b] + partial[2b+1] + bo.

Per-core kernel (all matmuls bf16 with fp32 psum accumulation):
  A) x [2048,1024] f32 -> PE-transpose -> xT [128,8dc,2048] bf16
  B) weights load+cast; v = x@Wv_h -> [128,16st,8h,65] bf16 (col 64 = ones,
     so the attention matmul also produces softmax denominators);
     qT/kT = (x@Wq_h).T layout [128(2 heads),4g,2048] bf16
  C) per head h, q-block j (512 q): for t-chunk c: sT=k@qT scores (causal-
     trimmed), exp on ScalarE (scale=1/8, no max-sub: |scores/8|<~3),
     upper-tri mask on the diagonal 128x128 block, ctx accumulation
     ctxT[65,512] += v_aug.T @ p. Row 64 = denominator. Normalize via
     K=1 ones-matmul broadcast + reciprocal_approx_fast + mul -> ctxT bf16.
  D) out = ctxT.T @ Wo_h -> [2048,1024] f32 partial output.
"""
import os
import sys

for _p in ("/opt/trn_rl_repo",):
    if os.path.isdir(_p) and _p not in sys.path:
        sys.path.insert(0, _p)

import numpy as np
from contextlib import ExitStack

import concourse.bass as bass
from concourse import bacc
import concourse.mybir as mybir
import concourse.tile as tile
from concourse import bass_utils
from concourse.masks import make_upper_triangular, make_identity

F32 = mybir.dt.float32
BF16 = mybir.dt.bfloat16
EXP = mybir.ActivationFunctionType.Exp

S = 2048          # sequence length
D = 1024          # d_model
P = 128           # partitions
NT = S // P       # 16 s-tiles
DC = D // P       # 8 d-model chunks
NH = 8            # heads per core
HD = 64           # head dim
NG = NH // 2      # head pairs (lhsT col groups of 128)
NJ = S // 512     # 4 q-blocks of 512
SCALE = HD ** -0.5

_CACHED_NC = None


def build_nc():
    nc = bacc.Bacc("TRN2", target_bir_lowering=False)
    x_d = nc.dram_tensor("x", (S, D), F32, kind="ExternalInput")
    wq_d = nc.dram_tensor("wq", (D, 512), F32, kind="ExternalInput")
    wk_d = nc.dram_tensor("wk", (D, 512), F32, kind="ExternalInput")
    wv_d = nc.dram_tensor("wv", (D, 512), F32, kind="ExternalInput")
    wo_d = nc.dram_tensor("wo", (512, D), F32, kind="ExternalInput")
    out_d = nc.dram_tensor("part", (S, D), F32, kind="ExternalOutput")

    with tile.TileContext(nc) as tc, ExitStack() as ctx:
        persist = ctx.enter_context(tc.tile_pool(name="persist", bufs=1))
        stage = ctx.enter_context(tc.tile_pool(name="stage", bufs=2))
        work = ctx.enter_context(tc.tile_pool(name="work", bufs=3))
        pT_pool = ctx.enter_context(tc.tile_pool(name="pT", bufs=4))
        norm_pool = ctx.enter_context(tc.tile_pool(name="norm", bufs=2))
        # PSUM budget: misc(2, shared ab+bc tags -> sized [128,512]) +
        # s pairs ([128,2,512] = 2 banks, bufs=2 -> 4) + ctx(2) = 8 banks
        ps_ab = ctx.enter_context(tc.tile_pool(name="ps_ab", bufs=2, space="PSUM"))
        ps_s = ctx.enter_context(tc.tile_pool(name="ps_s", bufs=2, space="PSUM"))
        ps_ctx = ctx.enter_context(tc.tile_pool(name="ps_ctx", bufs=2, space="PSUM"))

        # --- constants ---
        ident = persist.tile([P, P], F32)
        make_identity(nc, ident)
        tri = persist.tile([P, P], BF16)           # upper-tri incl diag (t<=q valid)
        make_upper_triangular(nc, tri, val=1.0, diag=True)
        ones1 = persist.tile([P, HD], F32)   # row 64 used as K=1 lhsT (base par 64)
        nc.vector.memset(ones1, 1.0)

        # --- persistent tensors ---
        xT = persist.tile([P, DC, S], BF16)        # [p, dc, s] : xT[dc*128+p, s]
        qT = persist.tile([P, NG, S], BF16)        # [p, g, s]  : row p = head-pair col
        kT = persist.tile([P, NG, S], BF16)
        v_all = persist.tile([P, NT, NH, HD + 1], BF16)
        ctxT = persist.tile([P, NG, S], BF16)

        # --- phase A: load x, transpose to xT ---
        for st in range(NT):
            x_f = work.tile([P, D], F32, tag="x_f")
            nc.sync.dma_start(x_f, x_d[st * P:(st + 1) * P, :])
            for dc in range(DC):
                tp = ps_ab.tile([P, 512], F32, tag="ab")
                nc.tensor.transpose(tp[:, 0:P], x_f[:, dc * P:(dc + 1) * P], ident)
                nc.vector.tensor_copy(xT[:, dc, st * P:(st + 1) * P], tp[:, 0:P])

        # --- phase B: weights ---
        wq_f = stage.tile([P, DC, 512], F32, tag="w_f")
        nc.sync.dma_start(wq_f, wq_d.rearrange("(dc p) c -> p dc c", p=P))
        wq_b = persist.tile([P, DC, 512], BF16)
        nc.vector.tensor_copy(wq_b, wq_f)
        wk_f = stage.tile([P, DC, 512], F32, tag="w_f")
        nc.sync.dma_start(wk_f, wk_d.rearrange("(dc p) c -> p dc c", p=P))
        wk_b = persist.tile([P, DC, 512], BF16)
        nc.vector.tensor_copy(wk_b, wk_f)
        wv_f = stage.tile([P, DC, 512], F32, tag="w_f")
        nc.sync.dma_start(wv_f, wv_d.rearrange("(dc p) c -> p dc c", p=P))
        wv_b = persist.tile([P, DC, 512], BF16)
        nc.vector.tensor_copy(wv_b, wv_f)
        wo_f = stage.tile([P, 4, D], F32, tag="w_f")
        nc.sync.dma_start(wo_f, wo_d.rearrange("(g p) e -> p g e", p=P))
        wo_b = persist.tile([P, 4, D], BF16)
        nc.vector.tensor_copy(wo_b, wo_f)

        # v = x @ Wv_h  (natural layout, strided by 65 with ones col)
        nc.vector.memset(v_all[:, :, :, HD], 1.0)
        for st in range(NT):
            pv = ps_ab.tile([P, 512], F32, tag="ab")
            for dc in range(DC):
                nc.tensor.matmul(pv, xT[:, dc, st * P:(st + 1) * P], wv_b[:, dc, :],
                                 start=(dc == 0), stop=(dc == DC - 1))
            nc.vector.tensor_copy(
                v_all[:, st, :, 0:HD],
                pv.rearrange("p (h e) -> p h e", h=NH))

        # --- phases B2+C interleaved per head-pair g ---
        for g in range(NG):
            # qT_g, kT_g projections
            for w_b, dstT in ((wq_b, qT), (wk_b, kT)):
                for sb in range(NJ):
                    pq = ps_ab.tile([P, 512], F32, tag="ab")
                    for dc in range(DC):
                        nc.tensor.matmul(
                            pq, w_b[:, dc, g * P:(g + 1) * P],
                            xT[:, dc, sb * 512:(sb + 1) * 512],
                            start=(dc == 0), stop=(dc == DC - 1))
                    nc.vector.tensor_copy(dstT[:, g, sb * 512:(sb + 1) * 512], pq)

            # attention for the two heads of pair g.
            # Chunk groups: full chunks in pairs (one [128,1024] exp each),
            # diagonal chunks single (trimmed + masked). The c-loop is
            # software-pipelined: scores/exp of group i are emitted before the
            # ctx matmuls of group i-1, so the PE never stalls on the ScalarE
            # exp and its matmuls pipeline back-to-back.
            for hh in range(2):
                h = 2 * g + hh
                row = hh * HD
                for j in range(NJ):
                    ctx_ps = ps_ctx.tile([HD + 1, 512], F32, tag="ctx")
                    groups = []
                    c = 0
                    while c < 4 * j:           # full chunks, paired
                        groups.append((c, c + 1) if c + 1 < 4 * j else (c,))
                        c += 2
                    for r in range(4):         # diagonal chunks, single
                        groups.append((4 * j + r,))

                    pend = None                # (chunks, pT, n0s)
                    first = True
                    for grp in groups:
                        s_ps = ps_s.tile([P, 2, 512], F32, tag="s")
                        pT = pT_pool.tile([P, 2, 512], BF16, tag="pT")
                        n0s = []
                        for sl, c in enumerate(grp):
                            r = c - 4 * j
                            n0 = 128 * r if r >= 0 else 0
                            n0s.append(n0)
                            nc.tensor.matmul(
                                s_ps[:, sl, n0:512],
                                kT[row:row + HD, g, c * P:(c + 1) * P],
                                qT[row:row + HD, g, j * 512 + n0:(j + 1) * 512],
                                start=True, stop=True)
                        if len(grp) == 2:
                            nc.scalar.activation(pT[:, :, :], s_ps[:, :, :],
                                                 EXP, scale=SCALE)
                        else:
                            n0 = n0s[0]
                            nc.scalar.activation(pT[:, 0, n0:512],
                                                 s_ps[:, 0, n0:512],
                                                 EXP, scale=SCALE)
                            if grp[0] - 4 * j >= 0:
                                nc.vector.tensor_mul(pT[:, 0, n0:n0 + P],
                                                     pT[:, 0, n0:n0 + P], tri)
                        if pend is not None:
                            for sl, (c, n0) in enumerate(zip(*pend[:2])):
                                nc.tensor.matmul(
                                    ctx_ps[:, n0:512],
                                    v_all[:, c, h, :], pend[2][:, sl, n0:512],
                                    start=first and sl == 0, stop=False)
                            first = False
                        pend = (grp, n0s, pT)
                    for sl, (c, n0) in enumerate(zip(*pend[:2])):
                        nc.tensor.matmul(
                            ctx_ps[:, n0:512],
                            v_all[:, c, h, :], pend[2][:, sl, n0:512],
                            start=first and sl == 0,
                            stop=(sl == len(pend[0]) - 1))
                    # normalization
                    raw = norm_pool.tile([HD + 1, 512], F32, tag="raw")
                    nc.vector.tensor_copy(raw, ctx_ps)
                    bc = ps_ab.tile([P, 512], F32, tag="ab")
                    nc.tensor.matmul(bc[0:HD, :], ones1[HD:HD + 1, :],
                                     raw[HD:HD + 1, :], start=True, stop=True)
                    rec = norm_pool.tile([HD, 512], F32, tag="rec")
                    nc.vector.reciprocal_approx_fast(rec, bc[0:HD, :])
                    nc.vector.tensor_tensor(
                        ctxT[row:row + HD, g, j * 512:(j + 1) * 512],
                        raw[0:HD, :], rec, mybir.AluOpType.mult)

        # --- phase D: out projection ---
        for st in range(NT):
            for eb in range(2):
                po = ps_ab.tile([P, 512], F32, tag="ab")
                for g in range(NG):
                    nc.tensor.matmul(
                        po, ctxT[:, g, st * P:(st + 1) * P],
                        wo_b[:, g, eb * 512:(eb + 1) * 512],
                        start=(g == 0), stop=(g == NG - 1))
                o_sb = work.tile([P, 512], F32, tag="o_sb")
                nc.vector.tensor_copy(o_sb, po)
                nc.sync.dma_start(out_d[st * P:(st + 1) * P, eb * 512:(eb + 1) * 512],
                                  o_sb)

    nc.compile()
    return nc


def _get_nc():
    global _CACHED_NC
    if _CACHED_NC is None:
        _CACHED_NC = build_nc()
    return _CACHED_NC


def _make_in_maps(x, Wq, Wk, Wv, Wo):
    in_maps = []
    for core in range(8):
        b, hg = core // 2, core % 2
        cs = slice(hg * 512, (hg + 1) * 512)
        in_maps.append({
            "x": np.ascontiguousarray(x[b]),
            "wq": np.ascontiguousarray(Wq[:, cs]),
            "wk": np.ascontiguousarray(Wk[:, cs]),
            "wv": np.ascontiguousarray(Wv[:, cs]),
            "wo": np.ascontiguousarray(Wo[cs, :]),
        })
    return in_maps


def run(x, Wq, Wk, Wv, Wo, bo, trace=False):
    nc = _get_nc()
    in_maps = _make_in_maps(x, Wq, Wk, Wv, Wo)
    res = bass_utils.run_bass_kernel_spmd(
        nc, in_maps, core_ids=list(range(8)), trace=trace)
    parts = [r["part"] for r in res.results]
    out = np.empty((4, S, D), dtype=np.float32)
    for b in range(4):
        out[b] = parts[2 * b] + parts[2 * b + 1]
    out += np.asarray(bo, dtype=np.float32)[None, None, :]
    return out, res


def kernel(x, Wq, Wk, Wv, Wo, bo):
    x = np.asarray(x, dtype=np.float32)
    Wq = np.asarray(Wq, dtype=np.float32)
    Wk = np.asarray(Wk, dtype=np.float32)
    Wv = np.asarray(Wv, dtype=np.float32)
    Wo = np.asarray(Wo, dtype=np.float32)
    bo = np.asarray(bo, dtype=np.float32)
    out, _ = run(x, Wq, Wk, Wv, Wo, bo, trace=False)
    return out


# revision 6
# speedup vs baseline: 1.0462x; 1.0462x over previous
"""Multi-head causal attention (B=4, S=2048, D=1024, H=16, hd=64) on 8 trn2 cores.

Sharding: core i handles batch b=i//2 and head-group hg=i%2 (8 heads).
Each core computes partial_out_b = ctx(heads of hg) @ Wo[rows of hg].
Host: out[b] = partial[2b] + partial[2b+1] + bo.

Per-core kernel (all matmuls bf16 with fp32 psum accumulation):
  A) x [2048,1024] f32 -> PE-transpose -> xT [128,8dc,2048] bf16
  B) weights load+cast; v = x@Wv_h -> [128,16st,8h,65] bf16 (col 64 = ones,
     so the attention matmul also produces softmax denominators);
     qT/kT = (x@Wq_h).T layout [128(2 heads),4g,2048] bf16
  C) per head h, q-block j (512 q): for t-chunk c: sT=k@qT scores (causal-
     trimmed), exp on ScalarE (scale=1/8, no max-sub: |scores/8|<~3),
     upper-tri mask on the diagonal 128x128 block, ctx accumulation
     ctxT[65,512] += v_aug.T @ p. Row 64 = denominator. Normalize via
     K=1 ones-matmul broadcast + reciprocal_approx_fast + mul -> ctxT bf16.
  D) out = ctxT.T @ Wo_h -> [2048,1024] f32 partial output.
"""
import os
import sys

for _p in ("/opt/trn_rl_repo",):
    if os.path.isdir(_p) and _p not in sys.path:
        sys.path.insert(0, _p)

import numpy as np
from contextlib import ExitStack

import concourse.bass as bass
from concourse import bacc
import concourse.mybir as mybir
import concourse.tile as tile
from concourse import bass_utils
from concourse.masks import make_upper_triangular, make_identity

F32 = mybir.dt.float32
BF16 = mybir.dt.bfloat16
EXP = mybir.ActivationFunctionType.Exp

S = 2048          # sequence length
D = 1024          # d_model
P = 128           # partitions
NT = S // P       # 16 s-tiles
DC = D // P       # 8 d-model chunks
NH = 8            # heads per core
HD = 64           # head dim
NG = NH // 2      # head pairs (lhsT col groups of 128)
NJ = S // 512     # 4 q-blocks of 512
SCALE = HD ** -0.5

_CACHED_NC = None


def build_nc():
    nc = bacc.Bacc("TRN2", target_bir_lowering=False)
    x_d = nc.dram_tensor("x", (S, D), F32, kind="ExternalInput")
    wq_d = nc.dram_tensor("wq", (D, 512), F32, kind="ExternalInput")
    wk_d = nc.dram_tensor("wk", (D, 512), F32, kind="ExternalInput")
    wv_d = nc.dram_tensor("wv", (D, 512), F32, kind="ExternalInput")
    wo_d = nc.dram_tensor("wo", (512, D), F32, kind="ExternalInput")
    out_d = nc.dram_tensor("part", (S, D), F32, kind="ExternalOutput")

    with tile.TileContext(nc) as tc, ExitStack() as ctx:
        persist = ctx.enter_context(tc.tile_pool(name="persist", bufs=1))
        stage = ctx.enter_context(tc.tile_pool(name="stage", bufs=2))
        work = ctx.enter_context(tc.tile_pool(name="work", bufs=3))
        pT_pool = ctx.enter_context(tc.tile_pool(name="pT", bufs=4))
        norm_pool = ctx.enter_context(tc.tile_pool(name="norm", bufs=2))
        # PSUM budget: misc(2, shared ab+bc tags -> sized [128,512]) +
        # s pairs ([128,2,512] = 2 banks, bufs=2 -> 4) + ctx(2) = 8 banks
        ps_ab = ctx.enter_context(tc.tile_pool(name="ps_ab", bufs=2, space="PSUM"))
        ps_s = ctx.enter_context(tc.tile_pool(name="ps_s", bufs=2, space="PSUM"))
        ps_ctx = ctx.enter_context(tc.tile_pool(name="ps_ctx", bufs=2, space="PSUM"))

        # --- constants ---
        ident = persist.tile([P, P], F32)
        make_identity(nc, ident)
        tri = persist.tile([P, P], BF16)           # upper-tri incl diag (t<=q valid)
        make_upper_triangular(nc, tri, val=1.0, diag=True)
        ones1 = persist.tile([P, HD], F32)   # row 64 used as K=1 lhsT (base par 64)
        nc.vector.memset(ones1, 1.0)

        # --- persistent tensors ---
        xT = persist.tile([P, DC, S], BF16)        # [p, dc, s] : xT[dc*128+p, s]
        qT = persist.tile([P, NG, S], BF16)        # [p, g, s]  : row p = head-pair col
        kT = persist.tile([P, NG, S], BF16)
        v_all = persist.tile([P, NT, NH, HD + 1], BF16)
        ctxT = persist.tile([P, NG, S], BF16)

        # --- phase A: load x, transpose to xT ---
        for st in range(NT):
            x_f = work.tile([P, D], F32, tag="x_f")
            nc.sync.dma_start(x_f, x_d[st * P:(st + 1) * P, :])
            for dc in range(DC):
                tp = ps_ab.tile([P, 512], F32, tag="ab")
                nc.tensor.transpose(tp[:, 0:P], x_f[:, dc * P:(dc + 1) * P], ident)
                nc.vector.tensor_copy(xT[:, dc, st * P:(st + 1) * P], tp[:, 0:P])

        # --- phase B: weights ---
        wq_f = stage.tile([P, DC, 512], F32, tag="w_f")
        nc.sync.dma_start(wq_f, wq_d.rearrange("(dc p) c -> p dc c", p=P))
        wq_b = persist.tile([P, DC, 512], BF16)
        nc.vector.tensor_copy(wq_b, wq_f)
        wk_f = stage.tile([P, DC, 512], F32, tag="w_f")
        nc.sync.dma_start(wk_f, wk_d.rearrange("(dc p) c -> p dc c", p=P))
        wk_b = persist.tile([P, DC, 512], BF16)
        nc.vector.tensor_copy(wk_b, wk_f)
        wv_f = stage.tile([P, DC, 512], F32, tag="w_f")
        nc.sync.dma_start(wv_f, wv_d.rearrange("(dc p) c -> p dc c", p=P))
        wv_b = persist.tile([P, DC, 512], BF16)
        nc.vector.tensor_copy(wv_b, wv_f)
        wo_f = stage.tile([P, 4, D], F32, tag="w_f")
        nc.sync.dma_start(wo_f, wo_d.rearrange("(g p) e -> p g e", p=P))
        wo_b = persist.tile([P, 4, D], BF16)
        nc.vector.tensor_copy(wo_b, wo_f)

        # v = x @ Wv_h  (natural layout, strided by 65 with ones col)
        nc.vector.memset(v_all[:, :, :, HD], 1.0)
        for st in range(NT):
            pv = ps_ab.tile([P, 512], F32, tag="ab")
            for dc in range(DC):
                nc.tensor.matmul(pv, xT[:, dc, st * P:(st + 1) * P], wv_b[:, dc, :],
                                 start=(dc == 0), stop=(dc == DC - 1))
            nc.vector.tensor_copy(
                v_all[:, st, :, 0:HD],
                pv.rearrange("p (h e) -> p h e", h=NH))

        # --- phase B2: all qT/kT projections (dense PE block, keeps HAM warm) ---
        for g in range(NG):
            for w_b, dstT in ((wq_b, qT), (wk_b, kT)):
                for sb in range(NJ):
                    pq = ps_ab.tile([P, 512], F32, tag="ab")
                    for dc in range(DC):
                        nc.tensor.matmul(
                            pq, w_b[:, dc, g * P:(g + 1) * P],
                            xT[:, dc, sb * 512:(sb + 1) * 512],
                            start=(dc == 0), stop=(dc == DC - 1))
                    nc.vector.tensor_copy(dstT[:, g, sb * 512:(sb + 1) * 512], pq)

        # --- phase C+D: attention (j outer, heads inner), out-proj per j ---
        # Chunk groups: full chunks in pairs (one [128,1024] exp each),
        # diagonal chunks single (trimmed + masked). The group loop is
        # software-pipelined: scores/exp of group i are emitted before the
        # ctx matmuls of group i-1, so the PE does not stall on the ScalarE
        # exp and its matmuls pipeline back-to-back. The out-projection for
        # q-block j runs right after all heads of j, giving the PE dense
        # independent work at block boundaries.
        for j in range(NJ):
            for h in range(NH):
                g, row = h // 2, (h % 2) * HD
                ctx_ps = ps_ctx.tile([HD + 1, 512], F32, tag="ctx")
                groups = []
                c = 0
                while c < 4 * j:           # full chunks, paired
                    groups.append((c, c + 1))
                    c += 2
                for r in range(4):         # diagonal chunks, single
                    groups.append((4 * j + r,))

                pend = None                # (chunks, n0s, pT)
                first = True
                for grp in groups:
                    s_ps = ps_s.tile([P, 2, 512], F32, tag="s")
                    pT = pT_pool.tile([P, 2, 512], BF16, tag="pT")
                    n0s = []
                    for sl, c in enumerate(grp):
                        r = c - 4 * j
                        n0 = 128 * r if r >= 0 else 0
                        n0s.append(n0)
                        nc.tensor.matmul(
                            s_ps[:, sl, n0:512],
                            kT[row:row + HD, g, c * P:(c + 1) * P],
                            qT[row:row + HD, g, j * 512 + n0:(j + 1) * 512],
                            start=True, stop=True)
                    if len(grp) == 2:
                        nc.scalar.activation(pT[:, :, :], s_ps[:, :, :],
                                             EXP, scale=SCALE)
                    else:
                        n0 = n0s[0]
                        nc.scalar.activation(pT[:, 0, n0:512],
                                             s_ps[:, 0, n0:512],
                                             EXP, scale=SCALE)
                        if grp[0] - 4 * j >= 0:
                            nc.vector.tensor_mul(pT[:, 0, n0:n0 + P],
                                                 pT[:, 0, n0:n0 + P], tri)
                    if pend is not None:
                        for sl, (c, n0) in enumerate(zip(pend[0], pend[1])):
                            nc.tensor.matmul(
                                ctx_ps[:, n0:512],
                                v_all[:, c, h, :], pend[2][:, sl, n0:512],
                                start=first and sl == 0, stop=False)
                        first = False
                    pend = (grp, n0s, pT)
                for sl, (c, n0) in enumerate(zip(pend[0], pend[1])):
                    nc.tensor.matmul(
                        ctx_ps[:, n0:512],
                        v_all[:, c, h, :], pend[2][:, sl, n0:512],
                        start=first and sl == 0,
                        stop=(sl == len(pend[0]) - 1))
                # normalization
                raw = norm_pool.tile([HD + 1, 512], F32, tag="raw")
                nc.vector.tensor_copy(raw, ctx_ps)
                bc = ps_ab.tile([P, 512], F32, tag="ab")
                nc.tensor.matmul(bc[0:HD, :], ones1[HD:HD + 1, :],
                                 raw[HD:HD + 1, :], start=True, stop=True)
                rec = norm_pool.tile([HD, 512], F32, tag="rec")
                nc.vector.reciprocal_approx_fast(rec, bc[0:HD, :])
                nc.vector.tensor_tensor(
                    ctxT[row:row + HD, g, j * 512:(j + 1) * 512],
                    raw[0:HD, :], rec, mybir.AluOpType.mult)

            # out projection for the 4 s-tiles of this q-block
            for st in range(4 * j, 4 * j + 4):
                for eb in range(2):
                    po = ps_ab.tile([P, 512], F32, tag="ab")
                    for g in range(NG):
                        nc.tensor.matmul(
                            po, ctxT[:, g, st * P:(st + 1) * P],
                            wo_b[:, g, eb * 512:(eb + 1) * 512],
                            start=(g == 0), stop=(g == NG - 1))
                    o_sb = work.tile([P, 512], F32, tag="o_sb")
                    nc.vector.tensor_copy(o_sb, po)
                    nc.sync.dma_start(
                        out_d[st * P:(st + 1) * P, eb * 512:(eb + 1) * 512], o_sb)

    nc.compile()
    return nc


def _get_nc():
    global _CACHED_NC
    if _CACHED_NC is None:
        _CACHED_NC = build_nc()
    return _CACHED_NC


def _make_in_maps(x, Wq, Wk, Wv, Wo):
    in_maps = []
    for core in range(8):
        b, hg = core // 2, core % 2
        cs = slice(hg * 512, (hg + 1) * 512)
        in_maps.append({
            "x": np.ascontiguousarray(x[b]),
            "wq": np.ascontiguousarray(Wq[:, cs]),
            "wk": np.ascontiguousarray(Wk[:, cs]),
            "wv": np.ascontiguousarray(Wv[:, cs]),
            "wo": np.ascontiguousarray(Wo[cs, :]),
        })
    return in_maps


def run(x, Wq, Wk, Wv, Wo, bo, trace=False):
    nc = _get_nc()
    in_maps = _make_in_maps(x, Wq, Wk, Wv, Wo)
    res = bass_utils.run_bass_kernel_spmd(
        nc, in_maps, core_ids=list(range(8)), trace=trace)
    parts = [r["part"] for r in res.results]
    out = np.empty((4, S, D), dtype=np.float32)
    for b in range(4):
        out[b] = parts[2 * b] + parts[2 * b + 1]
    out += np.asarray(bo, dtype=np.float32)[None, None, :]
    return out, res


def kernel(x, Wq, Wk, Wv, Wo, bo):
    x = np.asarray(x, dtype=np.float32)
    Wq = np.asarray(Wq, dtype=np.float32)
    Wk = np.asarray(Wk, dtype=np.float32)
    Wv = np.asarray(Wv, dtype=np.float32)
    Wo = np.asarray(Wo, dtype=np.float32)
    bo = np.asarray(bo, dtype=np.float32)
    out, _ = run(x, Wq, Wk, Wv, Wo, bo, trace=False)
    return out


# revision 7
# speedup vs baseline: 1.3212x; 1.2628x over previous
"""Multi-head causal attention (B=4, S=2048, D=1024, H=16, hd=64) on 8 trn2 cores.

Sharding: core i handles batch b=i//2 and head-group hg=i%2 (8 heads).
Each core computes partial_out_b = ctx(heads of hg) @ Wo[rows of hg].
Host: out[b] = partial[2b] + partial[2b+1] + bo.

Per-core kernel (all matmuls bf16 with fp32 psum accumulation), organized as a
software pipeline over q-blocks j (512 rows each). For each j: the loads,
PE-transposes, v- and qT/kT-projections for block j+1 are braided between the
attention heads of block j, so the TensorE always has dense independent work
while the ScalarE runs the exp stream (keeps the HAM clock gate warm).

Attention per (head, j), scores-transposed layout: for t-chunk c (128 rows):
sT = k @ qT (causal-trimmed), exp on ScalarE in chunk-pairs (scale=1/8, no
max-subtraction: |scores|/8 < ~3), upper-triangular mask on the diagonal
128x128 block, ctx accumulation ctxT[65,512] += v_aug.T @ p where v carries a
ones column so row 64 accumulates the softmax denominator. Normalization:
gpsimd partition_broadcast of the denominator row + DVE fast reciprocal + mul.
"""
import os
import sys

for _p in ("/opt/trn_rl_repo",):
    if os.path.isdir(_p) and _p not in sys.path:
        sys.path.insert(0, _p)

import numpy as np
from contextlib import ExitStack

import concourse.bass as bass
from concourse import bacc
import concourse.mybir as mybir
import concourse.tile as tile
from concourse import bass_utils
from concourse.masks import make_upper_triangular, make_identity

F32 = mybir.dt.float32
BF16 = mybir.dt.bfloat16
EXP = mybir.ActivationFunctionType.Exp

S = 2048          # sequence length
D = 1024          # d_model
P = 128           # partitions
NT = S // P       # 16 s-tiles
DC = D // P       # 8 d-model chunks
NH = 8            # heads per core
HD = 64           # head dim
NG = NH // 2      # head pairs (lhsT col groups of 128)
NJ = S // 512     # 4 q-blocks of 512
SCALE = HD ** -0.5

_CACHED_NC = None


def build_nc():
    nc = bacc.Bacc("TRN2", target_bir_lowering=False)
    x_d = nc.dram_tensor("x", (S, D), F32, kind="ExternalInput")
    wq_d = nc.dram_tensor("wq", (D, 512), F32, kind="ExternalInput")
    wk_d = nc.dram_tensor("wk", (D, 512), F32, kind="ExternalInput")
    wv_d = nc.dram_tensor("wv", (D, 512), F32, kind="ExternalInput")
    wo_d = nc.dram_tensor("wo", (512, D), F32, kind="ExternalInput")
    out_d = nc.dram_tensor("part", (S, D), F32, kind="ExternalOutput")

    with tile.TileContext(nc) as tc, ExitStack() as ctx:
        persist = ctx.enter_context(tc.tile_pool(name="persist", bufs=1))
        stage = ctx.enter_context(tc.tile_pool(name="stage", bufs=1))
        work = ctx.enter_context(tc.tile_pool(name="work", bufs=4))
        pT_pool = ctx.enter_context(tc.tile_pool(name="pT", bufs=4))
        norm_pool = ctx.enter_context(tc.tile_pool(name="norm", bufs=2))
        # PSUM: scores pairs 2x[128,2,512]=4 banks, prep 1x[128,2,512]=2,
        # ctx 2x[65,512]=2  -> 8 banks
        ps_s = ctx.enter_context(tc.tile_pool(name="ps_s", bufs=2, space="PSUM"))
        ps_prep = ctx.enter_context(tc.tile_pool(name="ps_prep", bufs=1, space="PSUM"))
        ps_ctx = ctx.enter_context(tc.tile_pool(name="ps_ctx", bufs=2, space="PSUM"))

        # --- constants ---
        ident = persist.tile([P, P], F32)
        make_identity(nc, ident)
        tri = persist.tile([P, P], BF16)           # upper-tri incl diag (t<=q valid)
        make_upper_triangular(nc, tri, val=1.0, diag=True)

        # --- persistent tensors ---
        xT = persist.tile([P, DC, S], BF16)        # [p, dc, s] : xT[dc*128+p, s]
        qT = persist.tile([P, NG, S], BF16)        # [p, g, s]
        kT = persist.tile([P, NG, S], BF16)
        v_all = persist.tile([P, NT, NH, HD + 1], BF16)
        ctxT = persist.tile([P, NG, S], BF16)
        wq_b = persist.tile([P, DC, 512], BF16)
        wk_b = persist.tile([P, DC, 512], BF16)
        wv_b = persist.tile([P, DC, 512], BF16)
        wo_b = persist.tile([P, 4, D], BF16)

        nc.vector.memset(v_all[:, :, :, HD], 1.0)

        def load_w(dram, dst_bf, shape, pat):
            w_f = stage.tile(shape, F32, tag="w_f")
            nc.sync.dma_start(w_f, dram.rearrange(pat, p=P))
            nc.vector.tensor_copy(dst_bf, w_f)

        def piece_tile(st):
            """Load x s-tile st, transpose into xT, compute v(st)."""
            x_f = work.tile([P, D], F32, tag="x_f")
            nc.sync.dma_start(x_f, x_d[st * P:(st + 1) * P, :])
            for half in range(2):
                tp = ps_prep.tile([P, 2, 512], F32, tag="prep")
                for q in range(4):
                    dc = half * 4 + q
                    nc.tensor.transpose(tp[:, 0, q * P:(q + 1) * P],
                                        x_f[:, dc * P:(dc + 1) * P], ident)
                nc.vector.tensor_copy(
                    xT[:, half * 4:half * 4 + 4, st * P:(st + 1) * P],
                    tp[:, 0, :].rearrange("p (dc q) -> p dc q", dc=4))
            pv = ps_prep.tile([P, 2, 512], F32, tag="prep")
            for dc in range(DC):
                nc.tensor.matmul(pv[:, 0, :], xT[:, dc, st * P:(st + 1) * P],
                                 wv_b[:, dc, :],
                                 start=(dc == 0), stop=(dc == DC - 1))
            nc.vector.tensor_copy(
                v_all[:, st, :, 0:HD],
                pv[:, 0, :].rearrange("p (h e) -> p h e", h=NH))

        def piece_qkT(w_b, dstT, g, sb):
            pq = ps_prep.tile([P, 2, 512], F32, tag="prep")
            for dc in range(DC):
                nc.tensor.matmul(
                    pq[:, 0, :], w_b[:, dc, g * P:(g + 1) * P],
                    xT[:, dc, sb * 512:(sb + 1) * 512],
                    start=(dc == 0), stop=(dc == DC - 1))
            nc.vector.tensor_copy(dstT[:, g, sb * 512:(sb + 1) * 512],
                                  pq[:, 0, :])

        def prep_pieces(sb):
            """Work pieces that prepare q-block sb (closures, emitted later)."""
            ps = []
            for st in range(4 * sb, 4 * sb + 4):
                ps.append(lambda st=st: piece_tile(st))
            for w_b, dstT in ((wq_b, qT), (wk_b, kT)):
                for g in range(NG):
                    ps.append(lambda w_b=w_b, dstT=dstT, g=g: piece_qkT(w_b, dstT, g, sb))
            return ps

        def attention(h, j):
            g, row = h // 2, (h % 2) * HD
            ctx_ps = ps_ctx.tile([HD + 1, 512], F32, tag="ctx")
            groups = []
            c = 0
            while c < 4 * j:               # full chunks, paired
                groups.append((c, c + 1))
                c += 2
            for r in range(4):             # diagonal chunks, single
                groups.append((4 * j + r,))

            pend = None                    # (chunks, n0s, pT)
            first = True
            for grp in groups:
                s_ps = ps_s.tile([P, 2, 512], F32, tag="s")
                pT = pT_pool.tile([P, 2, 512], BF16, tag="pT")
                n0s = []
                for sl, c in enumerate(grp):
                    r = c - 4 * j
                    n0 = 128 * r if r >= 0 else 0
                    n0s.append(n0)
                    nc.tensor.matmul(
                        s_ps[:, sl, n0:512],
                        kT[row:row + HD, g, c * P:(c + 1) * P],
                        qT[row:row + HD, g, j * 512 + n0:(j + 1) * 512],
                        start=True, stop=True)
                if len(grp) == 2:
                    nc.scalar.activation(pT[:, :, :], s_ps[:, :, :],
                                         EXP, scale=SCALE)
                else:
                    n0 = n0s[0]
                    nc.scalar.activation(pT[:, 0, n0:512], s_ps[:, 0, n0:512],
                                         EXP, scale=SCALE)
                    if grp[0] - 4 * j >= 0:
                        nc.vector.tensor_mul(pT[:, 0, n0:n0 + P],
                                             pT[:, 0, n0:n0 + P], tri)
                if pend is not None:
                    for sl, (c, n0) in enumerate(zip(pend[0], pend[1])):
                        nc.tensor.matmul(
                            ctx_ps[:, n0:512],
                            v_all[:, c, h, :], pend[2][:, sl, n0:512],
                            start=first and sl == 0, stop=False)
                    first = False
                pend = (grp, n0s, pT)
            for sl, (c, n0) in enumerate(zip(pend[0], pend[1])):
                nc.tensor.matmul(
                    ctx_ps[:, n0:512],
                    v_all[:, c, h, :], pend[2][:, sl, n0:512],
                    start=first and sl == 0, stop=(sl == len(pend[0]) - 1))
            # normalization: raw copy -> gpsimd broadcast of denom row ->
            # fast reciprocal -> multiply (all off the TensorE)
            raw = norm_pool.tile([HD + 1, 512], F32, tag="raw")
            nc.vector.tensor_copy(raw, ctx_ps)
            den_b = norm_pool.tile([HD, 512], F32, tag="den_b")
            nc.gpsimd.partition_broadcast(den_b, raw[HD:HD + 1, :])
            rec = norm_pool.tile([HD, 512], F32, tag="rec")
            nc.vector.reciprocal_approx_fast(rec, den_b)
            nc.vector.tensor_tensor(
                ctxT[row:row + HD, g, j * 512:(j + 1) * 512],
                raw[0:HD, :], rec, mybir.AluOpType.mult)

        def out_proj(j):
            for st in range(4 * j, 4 * j + 4):
                for eb in range(2):
                    po = ps_prep.tile([P, 2, 512], F32, tag="prep")
                    for g in range(NG):
                        nc.tensor.matmul(
                            po[:, 0, :], ctxT[:, g, st * P:(st + 1) * P],
                            wo_b[:, g, eb * 512:(eb + 1) * 512],
                            start=(g == 0), stop=(g == NG - 1))
                    o_sb = work.tile([P, 512], F32, tag="o_sb")
                    nc.vector.tensor_copy(o_sb, po[:, 0, :])
                    nc.sync.dma_start(
                        out_d[st * P:(st + 1) * P, eb * 512:(eb + 1) * 512], o_sb)

        # --- prologue: weights + q-block 0 prep (dense, warms the PE) ---
        load_w(wv_d, wv_b, [P, DC, 512], "(dc p) c -> p dc c")
        for piece in prep_pieces(0):
            piece()
        load_w(wq_d, wq_b, [P, DC, 512], "(dc p) c -> p dc c")
        load_w(wk_d, wk_b, [P, DC, 512], "(dc p) c -> p dc c")
        load_w(wo_d, wo_b, [P, 4, D], "(g p) e -> p g e")

        # --- main pipeline over q-blocks ---
        for j in range(NJ):
            pieces = prep_pieces(j + 1) if j + 1 < NJ else []
            for h in range(NH):
                attention(h, j)
                # braid next block's prep between heads (PE filler)
                k0 = (len(pieces) * h) // NH
                k1 = (len(pieces) * (h + 1)) // NH
                for piece in pieces[k0:k1]:
                    piece()
            out_proj(j)

    nc.compile()
    return nc


def _get_nc():
    global _CACHED_NC
    if _CACHED_NC is None:
        _CACHED_NC = build_nc()
    return _CACHED_NC


def _make_in_maps(x, Wq, Wk, Wv, Wo):
    in_maps = []
    for core in range(8):
        b, hg = core // 2, core % 2
        cs = slice(hg * 512, (hg + 1) * 512)
        in_maps.append({
            "x": np.ascontiguousarray(x[b]),
            "wq": np.ascontiguousarray(Wq[:, cs]),
            "wk": np.ascontiguousarray(Wk[:, cs]),
            "wv": np.ascontiguousarray(Wv[:, cs]),
            "wo": np.ascontiguousarray(Wo[cs, :]),
        })
    return in_maps


def run(x, Wq, Wk, Wv, Wo, bo, trace=False):
    nc = _get_nc()
    in_maps = _make_in_maps(x, Wq, Wk, Wv, Wo)
    res = bass_utils.run_bass_kernel_spmd(
        nc, in_maps, core_ids=list(range(8)), trace=trace)
    parts = [r["part"] for r in res.results]
    out = np.empty((4, S, D), dtype=np.float32)
    for b in range(4):
        out[b] = parts[2 * b] + parts[2 * b + 1]
    out += np.asarray(bo, dtype=np.float32)[None, None, :]
    return out, res


def kernel(x, Wq, Wk, Wv, Wo, bo):
    x = np.asarray(x, dtype=np.float32)
    Wq = np.asarray(Wq, dtype=np.float32)
    Wk = np.asarray(Wk, dtype=np.float32)
    Wv = np.asarray(Wv, dtype=np.float32)
    Wo = np.asarray(Wo, dtype=np.float32)
    bo = np.asarray(bo, dtype=np.float32)
    out, _ = run(x, Wq, Wk, Wv, Wo, bo, trace=False)
    return out


# revision 10
# speedup vs baseline: 1.4937x; 1.1306x over previous
"""Multi-head causal attention (B=4, S=2048, D=1024, H=16, hd=64) on 8 trn2 cores.

Sharding: core i handles batch b=i//2 and head-group hg=i%2 (8 heads).
Each core computes partial_out_b = ctx(heads of hg) @ Wo[rows of hg].
Host: out[b] = partial[2b] + partial[2b+1] + bo.

Per-core kernel (all matmuls bf16 with fp32 psum accumulation), organized as a
software pipeline over q-blocks j (512 rows each). For each j: the loads,
PE-transposes, v- and qT/kT-projections for block j+1 are braided between the
attention heads of block j, so the TensorE always has dense independent work
while the ScalarE runs the exp stream (keeps the HAM clock gate warm).

Attention per (head, j), scores-transposed layout: for t-chunk c (128 rows):
sT = k @ qT (causal-trimmed), exp on ScalarE in chunk-pairs (scale=1/8, no
max-subtraction: |scores|/8 < ~3), upper-triangular mask on the diagonal
128x128 block, ctx accumulation ctxT[65,512] += v_aug.T @ p where v carries a
ones column so row 64 accumulates the softmax denominator. Normalization:
gpsimd partition_broadcast of the denominator row + DVE fast reciprocal + mul.
"""
import os
import sys

for _p in ("/opt/trn_rl_repo",):
    if os.path.isdir(_p) and _p not in sys.path:
        sys.path.insert(0, _p)

import numpy as np
from contextlib import ExitStack

import concourse.bass as bass
from concourse import bacc
import concourse.mybir as mybir
import concourse.tile as tile
from concourse import bass_utils
from concourse.masks import make_upper_triangular, make_identity

F32 = mybir.dt.float32
BF16 = mybir.dt.bfloat16
EXP = mybir.ActivationFunctionType.Exp

S = 2048          # sequence length
D = 1024          # d_model
P = 128           # partitions
NT = S // P       # 16 s-tiles
DC = D // P       # 8 d-model chunks
NH = 8            # heads per core
HD = 64           # head dim
NG = NH // 2      # head pairs (lhsT col groups of 128)
NJ = S // 512     # 4 q-blocks of 512
SCALE = HD ** -0.5

_CACHED_NC = None


def build_nc():
    nc = bacc.Bacc("TRN2", target_bir_lowering=False)
    x_d = nc.dram_tensor("x", (S, D), F32, kind="ExternalInput")
    wq_d = nc.dram_tensor("wq", (D, 512), F32, kind="ExternalInput")
    wk_d = nc.dram_tensor("wk", (D, 512), F32, kind="ExternalInput")
    wv_d = nc.dram_tensor("wv", (D, 512), F32, kind="ExternalInput")
    wo_d = nc.dram_tensor("wo", (512, D), F32, kind="ExternalInput")
    out_d = nc.dram_tensor("part", (S, D), F32, kind="ExternalOutput")

    with tile.TileContext(nc) as tc, ExitStack() as ctx:
        persist = ctx.enter_context(tc.tile_pool(name="persist", bufs=1))
        stage = ctx.enter_context(tc.tile_pool(name="stage", bufs=1))
        work = ctx.enter_context(tc.tile_pool(name="work", bufs=4))
        pT_pool = ctx.enter_context(tc.tile_pool(name="pT", bufs=4))
        norm_pool = ctx.enter_context(tc.tile_pool(name="norm", bufs=2))
        # PSUM: scores pairs 2x[128,2,512]=4 banks, prep 1x[128,2,512]=2,
        # ctx 2x[65,512]=2  -> 8 banks
        ps_s = ctx.enter_context(tc.tile_pool(name="ps_s", bufs=2, space="PSUM"))
        ps_prep = ctx.enter_context(tc.tile_pool(name="ps_prep", bufs=1, space="PSUM"))
        ps_ctx = ctx.enter_context(tc.tile_pool(name="ps_ctx", bufs=2, space="PSUM"))

        # --- constants ---
        ident = persist.tile([P, P], F32)
        make_identity(nc, ident)
        tri = persist.tile([P, P], BF16)           # upper-tri incl diag (t<=q valid)
        make_upper_triangular(nc, tri, val=1.0, diag=True)

        # --- persistent tensors ---
        xT = persist.tile([P, DC, S], BF16)        # [p, dc, s] : xT[dc*128+p, s]
        qT = persist.tile([P, NG, S], BF16)        # [p, g, s]
        kT = persist.tile([P, NG, S], BF16)
        v_all = persist.tile([P, NT, NH, HD + 1], BF16)
        ctxT = persist.tile([P, NG, S], BF16)
        wq_b = persist.tile([P, DC, 512], BF16)
        wk_b = persist.tile([P, DC, 512], BF16)
        wv_b = persist.tile([P, DC, 512], BF16)
        wo_b = persist.tile([P, 4, D], BF16)

        nc.vector.memset(v_all[:, :, :, HD], 1.0)

        def load_w(dram, dst_bf, shape, pat):
            w_f = stage.tile(shape, F32, tag="w_f")
            nc.sync.dma_start(w_f, dram.rearrange(pat, p=P))
            nc.vector.tensor_copy(dst_bf, w_f)

        def piece_tile(st):
            """Load x s-tile st, transpose into xT, compute v(st)."""
            x_f = work.tile([P, D], F32, tag="x_f")
            nc.sync.dma_start(x_f, x_d[st * P:(st + 1) * P, :])
            for half in range(2):
                tp = ps_prep.tile([P, 2, 512], F32, tag="prep")
                for q in range(4):
                    dc = half * 4 + q
                    nc.tensor.transpose(tp[:, 0, q * P:(q + 1) * P],
                                        x_f[:, dc * P:(dc + 1) * P], ident)
                nc.vector.tensor_copy(
                    xT[:, half * 4:half * 4 + 4, st * P:(st + 1) * P],
                    tp[:, 0, :].rearrange("p (dc q) -> p dc q", dc=4))
            pv = ps_prep.tile([P, 2, 512], F32, tag="prep")
            for dc in range(DC):
                nc.tensor.matmul(pv[:, 0, :], xT[:, dc, st * P:(st + 1) * P],
                                 wv_b[:, dc, :],
                                 start=(dc == 0), stop=(dc == DC - 1))
            nc.vector.tensor_copy(
                v_all[:, st, :, 0:HD],
                pv[:, 0, :].rearrange("p (h e) -> p h e", h=NH))

        def piece_qkT(w_b, dstT, g, sb):
            pq = ps_prep.tile([P, 2, 512], F32, tag="prep")
            for dc in range(DC):
                nc.tensor.matmul(
                    pq[:, 0, :], w_b[:, dc, g * P:(g + 1) * P],
                    xT[:, dc, sb * 512:(sb + 1) * 512],
                    start=(dc == 0), stop=(dc == DC - 1))
            nc.vector.tensor_copy(dstT[:, g, sb * 512:(sb + 1) * 512],
                                  pq[:, 0, :])

        def prep_pieces(sb):
            """Work pieces that prepare q-block sb (closures, emitted later)."""
            ps = []
            for st in range(4 * sb, 4 * sb + 4):
                ps.append(lambda st=st: piece_tile(st))
            for w_b, dstT in ((wq_b, qT), (wk_b, kT)):
                for g in range(NG):
                    ps.append(lambda w_b=w_b, dstT=dstT, g=g: piece_qkT(w_b, dstT, g, sb))
            return ps

        def attention_pair(g, j):
            """Both heads of pair g at once: their K=64 score matmuls are
            issued back-to-back to PE row-groups 0-1 / 2-3 (base partitions
            0 / 64) so they run concurrently; one exp ACTIVATE covers both."""
            ctx_ab = [ps_ctx.tile([HD + 1, 512], F32, tag="ctx",
                                  name=f"ctx_{g}_{j}_{hh}")
                      for hh in range(2)]
            nchunks = 4 * j + 4
            pend = None                    # (c, n0, pT)
            first = True

            def ctx_mms(c, n0, pT, last):
                nonlocal first
                for hh in range(2):
                    nc.tensor.matmul(
                        ctx_ab[hh][:, n0:512],
                        v_all[:, c, 2 * g + hh, :], pT[:, hh, n0:512],
                        start=first, stop=last)
                first = False

            for c in range(nchunks):
                r = c - 4 * j
                n0 = 128 * r if r >= 0 else 0
                s_ps = ps_s.tile([P, 2, 512], F32, tag="s")
                pT = pT_pool.tile([P, 2, 512], BF16, tag="pT")
                for hh in range(2):
                    row = hh * HD
                    nc.tensor.matmul(
                        s_ps[:, hh, n0:512],
                        kT[row:row + HD, g, c * P:(c + 1) * P],
                        qT[row:row + HD, g, j * 512 + n0:(j + 1) * 512],
                        start=True, stop=True)
                nc.scalar.activation(pT[:, :, n0:512], s_ps[:, :, n0:512],
                                     EXP, scale=SCALE)
                if r >= 0:
                    for hh in range(2):
                        nc.vector.tensor_mul(pT[:, hh, n0:n0 + P],
                                             pT[:, hh, n0:n0 + P], tri)
                if pend is not None:
                    ctx_mms(pend[0], pend[1], pend[2], last=False)
                pend = (c, n0, pT)
            ctx_mms(pend[0], pend[1], pend[2], last=True)

            # normalization: raw copy -> gpsimd broadcast of denom row ->
            # fast reciprocal -> multiply (all off the TensorE)
            for hh in range(2):
                row = hh * HD
                raw = norm_pool.tile([HD + 1, 512], F32, tag="raw")
                nc.vector.tensor_copy(raw, ctx_ab[hh])
                den_b = norm_pool.tile([HD, 512], F32, tag="den_b")
                nc.gpsimd.partition_broadcast(den_b, raw[HD:HD + 1, :])
                rec = norm_pool.tile([HD, 512], F32, tag="rec")
                nc.vector.reciprocal_approx_fast(rec, den_b)
                nc.vector.tensor_tensor(
                    ctxT[row:row + HD, g, j * 512:(j + 1) * 512],
                    raw[0:HD, :], rec, mybir.AluOpType.mult)

        def out_proj(j):
            for st in range(4 * j, 4 * j + 4):
                for eb in range(2):
                    po = ps_prep.tile([P, 2, 512], F32, tag="prep")
                    for g in range(NG):
                        nc.tensor.matmul(
                            po[:, 0, :], ctxT[:, g, st * P:(st + 1) * P],
                            wo_b[:, g, eb * 512:(eb + 1) * 512],
                            start=(g == 0), stop=(g == NG - 1))
                    o_sb = work.tile([P, 512], F32, tag="o_sb")
                    nc.vector.tensor_copy(o_sb, po[:, 0, :])
                    nc.sync.dma_start(
                        out_d[st * P:(st + 1) * P, eb * 512:(eb + 1) * 512], o_sb)

        # --- prologue: wv + x tiles of block 0 + wq/wk, then only g=0
        # projections, so attention starts as early as possible ---
        load_w(wv_d, wv_b, [P, DC, 512], "(dc p) c -> p dc c")
        for st in range(4):
            piece_tile(st)
        load_w(wq_d, wq_b, [P, DC, 512], "(dc p) c -> p dc c")
        load_w(wk_d, wk_b, [P, DC, 512], "(dc p) c -> p dc c")
        piece_qkT(wq_b, qT, 0, 0)
        piece_qkT(wk_b, kT, 0, 0)

        # --- main pipeline over q-blocks; attention per head-pair with the
        # next block's prep (and j=0: remaining g's projections) braided in
        # as dense PE filler ---
        for j in range(NJ):
            pieces = prep_pieces(j + 1) if j + 1 < NJ else []
            if j == 0:
                pieces = pieces + [lambda: load_w(wo_d, wo_b, [P, 4, D],
                                                  "(g p) e -> p g e")]
            for g in range(NG):
                attention_pair(g, j)
                if j == 0 and g < NG - 1:
                    # next pair's projections must be emitted before its use
                    piece_qkT(wq_b, qT, g + 1, 0)
                    piece_qkT(wk_b, kT, g + 1, 0)
                k0 = (len(pieces) * g) // NG
                k1 = (len(pieces) * (g + 1)) // NG
                for piece in pieces[k0:k1]:
                    piece()
            out_proj(j)

    nc.compile()
    return nc


def _get_nc():
    global _CACHED_NC
    if _CACHED_NC is None:
        _CACHED_NC = build_nc()
    return _CACHED_NC


def _make_in_maps(x, Wq, Wk, Wv, Wo):
    in_maps = []
    for core in range(8):
        b, hg = core // 2, core % 2
        cs = slice(hg * 512, (hg + 1) * 512)
        in_maps.append({
            "x": np.ascontiguousarray(x[b]),
            "wq": np.ascontiguousarray(Wq[:, cs]),
            "wk": np.ascontiguousarray(Wk[:, cs]),
            "wv": np.ascontiguousarray(Wv[:, cs]),
            "wo": np.ascontiguousarray(Wo[cs, :]),
        })
    return in_maps


def run(x, Wq, Wk, Wv, Wo, bo, trace=False):
    nc = _get_nc()
    in_maps = _make_in_maps(x, Wq, Wk, Wv, Wo)
    res = bass_utils.run_bass_kernel_spmd(
        nc, in_maps, core_ids=list(range(8)), trace=trace)
    parts = [r["part"] for r in res.results]
    out = np.empty((4, S, D), dtype=np.float32)
    for b in range(4):
        out[b] = parts[2 * b] + parts[2 * b + 1]
    out += np.asarray(bo, dtype=np.float32)[None, None, :]
    return out, res


def kernel(x, Wq, Wk, Wv, Wo, bo):
    x = np.asarray(x, dtype=np.float32)
    Wq = np.asarray(Wq, dtype=np.float32)
    Wk = np.asarray(Wk, dtype=np.float32)
    Wv = np.asarray(Wv, dtype=np.float32)
    Wo = np.asarray(Wo, dtype=np.float32)
    bo = np.asarray(bo, dtype=np.float32)
    out, _ = run(x, Wq, Wk, Wv, Wo, bo, trace=False)
    return out
